# revision 16
# baseline (speedup 1.0000x reference)
"""ArcFace loss kernel for 8 Trainium2 NeuronCores (Bass/Tile).

out = S * clip(emb @ (kernel / ||kernel||_col), -1, 1), with out[i, label[i]]
replaced by S * (cos*cos_m - sin*sin_m).

Sharding: class (column) dim split across 8 cores, 12800 padded columns each
(100000 -> 102400, pad columns = 1.0, dropped on gather). Embeddings are
replicated. No inter-core communication is needed.

Per-core device graph:
  - normalize: bcast = ones(1,128).T @ inv_row (K=1 matmul -> PSUM broadcast
    of S/||k_j|| across partitions), kn = kernel_shard * bcast.
  - 16 row-tiles x 25 col-chunks of matmul (float32r, full PE rate),
    eviction PSUM->SBUF fused with the *S clip via one tensor_scalar(min,max).
  - bulk DMA out per (row-tile, 2560-col group).
  - label patch: indirect-DMA gather of out[i, label_i] (only rows whose label
    falls in this core's shard; others bounds-checked out), compute the
    margin-corrected value on-chip, indirect-DMA scatter it back.
"""

import math
import os

import numpy as np

import concourse.bacc as bacc
import concourse.bass as bass
import concourse.mybir as mybir
import concourse.tile as tile
from concourse.bass_utils import run_bass_kernel_spmd
from concourse.tile_rust import add_dep_helper

EMBED = 128
CLASSNUM = 100000
NB = 2048
S = 64.0
MARGIN = 0.5
COS_M = math.cos(MARGIN)
SIN_M = math.sin(MARGIN)

NCORES = 8
CPAD = 102400           # padded class count (divisible by 8*512)
PER = CPAD // NCORES    # 12800 columns per core
CHUNK = 512             # matmul moving dim / PSUM bank
NCHUNKS = PER // CHUNK  # 25
RTILES = NB // 128      # 16 row tiles
STAGE = 2560            # columns per staging buffer / bulk DMA
NSTAGE = PER // STAGE   # 5
CPS = STAGE // CHUNK    # 5 chunks per staging group
SENTINEL = 1 << 28      # out-of-shard flat index (dropped by bounds_check)

LAST_EXEC_NS = None
LAST_TRACE = None

_CACHED_NC = None


def _install_profile_hook_shim():
    """bass_utils imports antenv.axon_hooks for trace=True under axon; this
    environment's antenv lacks that module. Provide it and register the
    ctypes-based NTFF hook from trn_agent_boot."""
    import sys
    import types
    try:
        import antenv.axon_hooks  # noqa: F401
        return
    except ImportError:
        pass
    mod = types.ModuleType("antenv.axon_hooks")
    holder = [None]
    mod.set_axon_ntff_profile_hook = lambda h: holder.__setitem__(0, h)
    mod.get_axon_ntff_profile_hook = lambda: holder[0]
    sys.modules["antenv.axon_hooks"] = mod
    import antenv
    antenv.axon_hooks = mod
    try:
        from trn_agent_boot.trn_boot import _ntff_profile_via_ctypes
        hook = _ntff_profile_via_ctypes("/opt/axon/libaxon_pjrt.so")
        if hook is not None:
            mod.set_axon_ntff_profile_hook(hook)
    except Exception:
        pass


def _build_nc():
    f32 = mybir.dt.float32
    f32r = mybir.dt.float32r
    i32 = mybir.dt.int32

    # Bacc (not Bass): its finalize() runs compile(), which legalizes
    # multi-wait instructions (TRN2 allows 1 sync wait per instruction).
    nc = bacc.Bacc()
    embT_ext = nc.declare_dram_parameter("embT", [EMBED, NB], f32, isOutput=False)
    ksh_ext = nc.declare_dram_parameter("ksh", [EMBED, PER], f32, isOutput=False)
    # [1, 128] of ones (bcast-matmul lhsT) ++ [1, PER] of S/norm
    inv_ext = nc.declare_dram_parameter("invrow", [1, 128 + PER], f32, isOutput=False)
    offs_ext = nc.declare_dram_parameter("offs", [128, RTILES], i32, isOutput=False)
    out_ext = nc.declare_dram_parameter("out", [NB, PER], f32, isOutput=True)

    with tile.TileContext(nc) as tc:
        with (
            tc.tile_pool(name="big", bufs=1) as big,
            tc.tile_pool(name="stage", bufs=3) as stg,
            tc.tile_pool(name="small", bufs=1) as small,
            tc.tile_pool(name="psum", bufs=6, space="PSUM") as pp,
            tc.tile_pool(name="bpsum", bufs=2, space="PSUM") as bp,
        ):
            embT = big.tile([EMBED, NB], f32)
            nc.sync.dma_start(out=embT[:], in_=embT_ext[:])
            ksh = big.tile([EMBED, PER], f32)
            nc.sync.dma_start(out=ksh[:], in_=ksh_ext[:])
            invr = big.tile([1, 128 + PER], f32)
            nc.sync.dma_start(out=invr[:], in_=inv_ext[:])
            offs = big.tile([128, RTILES], i32)
            nc.sync.dma_start(out=offs[:], in_=offs_ext[:])

            ones = invr[:, 0:128]

            # walrus allows only one sync-wait on the LDWEIGHTS+MATMUL pair,
            # so each matmul's operands must come from a single producer
            # engine: the bcast matmuls read only the invr DMA; the big
            # matmuls read only DVE-written tiles (embR, kn).
            # embeddings rounded to f32r for the full-rate PE path
            embR = big.tile([EMBED, NB], f32r)
            nc.vector.tensor_copy(embR[:], embT[:])

            # kn[:, c] = ksh[:, c] * (S / norm_c), broadcast along partitions
            kn = big.tile([EMBED, PER], f32r)
            for c in range(NCHUNKS):
                cs = slice(c * CHUNK, (c + 1) * CHUNK)
                bc = bp.tile([128, CHUNK], f32)
                nc.tensor.matmul(
                    bc[:], ones, invr[:, 128 + c * CHUNK:128 + (c + 1) * CHUNK],
                    start=True, stop=True)
                nc.vector.scalar_tensor_tensor(
                    kn[:, cs], bc[:], 1.0, ksh[:, cs],
                    op0=mybir.AluOpType.mult, op1=mybir.AluOpType.mult,
                )

            # main matmul + clip-evict + bulk DMA out
            bulk_dmas = []
            for m in range(RTILES):
                emb_m = embR[:, m * 128:(m + 1) * 128]
                for g in range(NSTAGE):
                    st = stg.tile([128, STAGE], f32)
                    for cc in range(CPS):
                        c = g * CPS + cc
                        ps = pp.tile([128, CHUNK], f32)
                        nc.tensor.matmul(
                            ps[:], emb_m,
                            kn[:, c * CHUNK:(c + 1) * CHUNK],
                            start=True, stop=True,
                        )
                        nc.vector.tensor_scalar(
                            st[:, cc * CHUNK:(cc + 1) * CHUNK], ps[:],
                            S, -S,
                            op0=mybir.AluOpType.min, op1=mybir.AluOpType.max,
                        )
                    d = nc.sync.dma_start(
                        out=out_ext[m * 128:(m + 1) * 128,
                                    g * STAGE:(g + 1) * STAGE],
                        in_=st[:],
                    )
                    bulk_dmas.append(d)

            # label patch: gather diag values, compute margin value, scatter
            # back. Indirect DMA consumes ONE offset per partition (free dims
            # iterate within it), so one (128,1) gather/scatter per row tile.
            gsrc = small.tile([128, RTILES], f32)
            nc.vector.memset(gsrc[:], 0.0)
            gths = []
            for m in range(RTILES):
                gth = nc.gpsimd.indirect_dma_start(
                    out=gsrc[:, m:m + 1],
                    out_offset=None,
                    in_=out_ext[:, :],
                    in_offset=bass.IndirectOffsetOnAxis(ap=offs[:, m:m + 1], axis=1),
                    bounds_check=NB * PER - 1,
                    oob_is_err=False,
                )
                for d in bulk_dmas[m * NSTAGE:(m + 1) * NSTAGE]:
                    add_dep_helper(gth.ins, d.ins, True, "gather after bulk out")
                gths.append(gth)

            v = small.tile([128, RTILES], f32)
            nc.vector.tensor_scalar(
                v[:], gsrc[:], 1.0 / S, None, op0=mybir.AluOpType.mult)
            v2 = small.tile([128, RTILES], f32)
            nc.vector.tensor_tensor(
                out=v2[:], in0=v[:], in1=v[:], op=mybir.AluOpType.mult)
            om = small.tile([128, RTILES], f32)
            nc.vector.tensor_scalar(
                om[:], v2[:], -1.0, 1.0,
                op0=mybir.AluOpType.mult, op1=mybir.AluOpType.add)
            sn = small.tile([128, RTILES], f32)
            nc.scalar.sqrt(sn[:], om[:])
            t1 = small.tile([128, RTILES], f32)
            nc.vector.tensor_scalar(
                t1[:], v[:], S * COS_M, None, op0=mybir.AluOpType.mult)
            corr = small.tile([128, RTILES], f32)
            nc.vector.scalar_tensor_tensor(
                corr[:], sn[:], -S * SIN_M, t1[:],
                op0=mybir.AluOpType.mult, op1=mybir.AluOpType.add,
            )
            for m in range(RTILES):
                sct = nc.gpsimd.indirect_dma_start(
                    out=out_ext[:, :],
                    out_offset=bass.IndirectOffsetOnAxis(ap=offs[:, m:m + 1], axis=1),
                    in_=corr[:, m:m + 1],
                    in_offset=None,
                    bounds_check=NB * PER - 1,
                    oob_is_err=False,
                )
                for g in gths:
                    add_dep_helper(sct.ins, g.ins, True, "scatter after gathers")
    nc.finalize()
    return nc


def _get_nc():
    global _CACHED_NC
    if _CACHED_NC is None:
        _CACHED_NC = _build_nc()
    return _CACHED_NC


def kernel(embbedings, label, kernel):
    global LAST_EXEC_NS, LAST_TRACE
    emb = np.ascontiguousarray(np.asarray(embbedings, dtype=np.float32))
    ker = np.asarray(kernel, dtype=np.float32)
    lab = np.asarray(label).astype(np.int64)
    assert emb.shape == (NB, EMBED) and ker.shape == (EMBED, CLASSNUM)

    embT = np.ascontiguousarray(emb.T)
    inv = (S / np.sqrt((ker.astype(np.float64) ** 2).sum(axis=0))).astype(np.float32)
    inv_pad = np.concatenate([inv, np.full(CPAD - CLASSNUM, 1.0, np.float32)])
    ker_pad = np.concatenate(
        [ker, np.ones((EMBED, CPAD - CLASSNUM), np.float32)], axis=1)

    rows = np.arange(NB, dtype=np.int64)
    in_maps = []
    for c in range(NCORES):
        c0 = c * PER
        lloc = lab - c0
        inrange = (lloc >= 0) & (lloc < PER)
        flat = np.where(inrange, rows * PER + lloc, SENTINEL)
        offs = np.ascontiguousarray(
            flat.reshape(RTILES, 128).T.astype(np.int32))
        invrow = np.concatenate(
            [np.ones(128, np.float32), inv_pad[c0:c0 + PER]]).reshape(1, -1)
        in_maps.append({
            "embT": embT,
            "ksh": np.ascontiguousarray(ker_pad[:, c0:c0 + PER]),
            "invrow": np.ascontiguousarray(invrow),
            "offs": offs,
        })

    nc = _get_nc()
    trace = os.environ.get("ARCFACE_TRACE", "") == "1"
    if trace:
        _install_profile_hook_shim()
    res = run_bass_kernel_spmd(
        nc, in_maps, core_ids=list(range(NCORES)), trace=trace)
    LAST_EXEC_NS = res.exec_time_ns
    LAST_TRACE = getattr(res, "instructions_and_trace", None)
    out = np.concatenate(
        [res.results[i]["out"] for i in range(NCORES)], axis=1)[:, :CLASSNUM]
    return np.ascontiguousarray(out)
